# revision 20
# baseline (speedup 1.0000x reference)
# Distributed Trainium2 kernel for nn_DecoderRNN (attention decoder RNN).
#
# Sharding: encoder T axis sharded 8-way for the per-step attention
# (partials AllGathered each step and reduced on-chip with a ones-matmul);
# GRU replicated on all cores; Wout sharded over vocab (2000/core).
# All matmuls run in bf16 with f32 PSUM accumulation. Activations that feed
# contractions over H are produced directly in transposed [H, B] layout to
# avoid per-step transposes where possible.
#
# Biases: br is folded into the precomputed per-step rhs (exact); bout is
# added host-side; bih*/bhh* are zeros by the problem spec (fill: zeros).

import numpy as np

NCORES = 8
B, T, TDEC, H, V = 32, 64, 48, 512, 16000
TL = T // NCORES          # 8 encoder timesteps per core
BT = B * TL               # 256 local (b, t) rows
VS = V // NCORES          # 2000 vocab entries per core
HK = H // 128             # 4 contraction k-tiles

_BUILT = None


def _build():
    from concourse import bacc, mybir
    from concourse import tile
    from concourse.bass import broadcast_tensor_aps

    f32 = mybir.dt.float32
    bf16 = mybir.dt.bfloat16
    AX = mybir.AxisListType
    OP = mybir.AluOpType
    AF = mybir.ActivationFunctionType

    nc = bacc.Bacc("TRN2", target_bir_lowering=False, num_devices=NCORES)

    dram_in = lambda name, shape, dt: nc.dram_tensor(name, shape, dt, kind="ExternalInput")
    # ---- external inputs (per-core shards prepared host-side) ----
    d_encT = dram_in("encT", [128, HK, BT], bf16)      # enc^T shard  [h, (b,tl)]
    d_encbt = dram_in("encbt", [128, 2, H], bf16)      # enc shard    [(b,tl), h]
    d_w12t = dram_in("w12t", [128, HK, H], bf16)       # (W1+W2)^T
    d_w1t = dram_in("w1t", [128, HK, H], bf16)         # W1^T
    d_vtt = dram_in("vtt", [128, HK, H], bf16)         # vT^T
    d_w2t = dram_in("w2t", [128, HK, H], bf16)         # W2^T
    d_wih = dram_in("wih", [128, HK, 6 * H], bf16)     # [Wih1^T | Wih2^T]
    d_whh1 = dram_in("whh1", [128, HK, 3 * H], bf16)   # Whh1^T
    d_whh2 = dram_in("whh2", [128, HK, 3 * H], bf16)   # Whh2^T
    d_wout = dram_in("wout", [128, HK, VS], bf16)      # Wout^T vocab shard
    d_rhsx = dram_in("rhsx", [33, TDEC, B], bf16)      # [Wr_r^T ; Xl_t + br]
    d_maskl = dram_in("maskl", [128, 2, B], bf16)      # local t-sum masks
    d_maskg = dram_in("maskg", [128, 2, B], bf16)      # gathered core-sum masks
    d_rhsg = dram_in("rhsg", [128, 2, B], bf16)        # WrR^T expanded to gathered rows
    d_id32 = dram_in("id32", [128, 128], f32)
    d_id16 = dram_in("id16", [128, 128], bf16)

    d_out = nc.dram_tensor("out", [TDEC * B, VS], bf16, kind="ExternalOutput")
    d_state = nc.dram_tensor("state_out", [B, H], f32, kind="ExternalOutput")

    # ---- internal DRAM (collective bounce ring) ----
    bnc_in = [nc.dram_tensor(f"agin{i}", [B, H], bf16) for i in range(2)]
    bnc_out = [
        nc.dram_tensor(f"agout{i}", [128, 2, H], bf16, addr_space="Shared")
        for i in range(2)
    ]

    # ---- persistent SBUF ----
    sb = nc.alloc_sbuf_tensor
    encT = sb("s_encT", [128, HK, BT], bf16)
    encbt = sb("s_encbt", [128, 2, H], bf16)
    w12t = sb("s_w12t", [128, HK, H], bf16)
    w1tw = sb("s_w1tw", [128, HK, H], bf16)
    vtt = sb("s_vtt", [128, HK, H], bf16)
    w2t = sb("s_w2t", [128, HK, H], bf16)
    wih = sb("s_wih", [128, HK, 6 * H], bf16)
    whh1 = sb("s_whh1", [128, HK, 3 * H], bf16)
    whh2 = sb("s_whh2", [128, HK, 3 * H], bf16)
    wout = sb("s_wout", [128, HK, VS], bf16)
    rhsx = sb("s_rhsx", [33, TDEC, B], bf16)
    maskl = sb("s_maskl", [128, 2, B], bf16)
    maskg = sb("s_maskg", [128, 2, B], bf16)
    rhsg = sb("s_rhsg", [128, 2, B], bf16)
    id32 = sb("s_id32", [128, 128], f32)
    id16 = sb("s_id16", [128, 128], bf16)
    w1ta = sb("s_w1ta", [128, HK, TL, B], f32)   # w1^T acts, t-major free dims
    xt_lhs = sb("s_xtlhs", [33, H], bf16)        # rows 0-31: attns, row 32: ones
    xt_sb = sb("s_xt", [128, HK, B], bf16)       # x^T
    st1T = sb("s_st1T", [128, HK, B], bf16)      # state1^T
    st2T = sb("s_st2T", [128, HK, B], bf16)      # state2^T (carry)
    state = sb("s_state", [B, H], f32)           # carry state (normal layout)
    zt = sb("s_zt", [128, HK, BT], bf16)         # tanh(w1+q2)^T
    gath = sb("s_gath", [128, 2, H], bf16)       # gathered partials

    MM = nc.tensor.matmul
    TP = nc.tensor.transpose
    ACT = nc.scalar.activation
    rg = [list(range(NCORES))]

    with tile.TileContext(nc) as tc:
        with (
            tc.tile_pool(name="sp", bufs=2) as sp,
            tc.tile_pool(name="pp1", bufs=1, space="PSUM") as pp1,
            tc.tile_pool(name="pp2", bufs=1, space="PSUM") as pp2,
        ):
            # ---------- preload ----------
            # engines used purely as DMA triggers
            E = [nc.sync, nc.scalar, nc.gpsimd, nc.sync]
            loads = [
                (encT, d_encT), (w12t, d_w12t), (w1tw, d_w1t), (encbt, d_encbt),
                (vtt, d_vtt), (w2t, d_w2t), (rhsx, d_rhsx), (maskl, d_maskl),
                (maskg, d_maskg), (id32, d_id32), (id16, d_id16), (rhsg, d_rhsg),
            ]
            for i, (dst, src) in enumerate(loads):
                E[i % 4].dma_start(dst.ap(), src.ap())
            # big weights, chunked across engines
            for k in range(HK):
                nc.sync.dma_start(wih[:, k, :], d_wih[:, k, :])
                E[k % 4].dma_start(whh1[:, k, :], d_whh1[:, k, :])
                E[(k + 1) % 4].dma_start(whh2[:, k, :], d_whh2[:, k, :])
                nc.scalar.dma_start(wout[:, k, :], d_wout[:, k, :])

            nc.gpsimd.memset(xt_lhs[32:33, :], 1.0)
            nc.gpsimd.memset(state[:, :], 0.0)
            nc.gpsimd.memset(st2T[:, :, :], 0.0)

            # ---------- precompute w1^T acts and z0 ----------
            for m in range(HK):
                ps_w = pp1.tile([128, BT], f32, tag="A")
                for k in range(HK):
                    MM(ps_w[:, :], w1tw[:, k, 128 * m:128 * m + 128], encT[:, k, :],
                       start=(k == 0), stop=(k == HK - 1))
                ACT(w1ta[:, m, :, :], ps_w[:, :], AF.Copy)
            for m in range(HK):
                ps_z = pp1.tile([128, BT], f32, tag="C")
                for k in range(HK):
                    MM(ps_z[:, :], w12t[:, k, 128 * m:128 * m + 128], encT[:, k, :],
                       start=(k == 0), stop=(k == HK - 1))
                ACT(zt[:, m, :], ps_z[:, :], AF.Tanh)

            sa_tiles = {}

            def attend_A(j, with_q2):
                """q2^T -> z^T -> u -> softmax -> local partial -> AG trigger."""
                if with_q2:
                    ps_q2 = pp1.tile([128, HK, B], f32, tag="small")
                    for m in range(HK):
                        for k in range(HK):
                            MM(ps_q2[:, m, :], w2t[:, k, 128 * m:128 * m + 128],
                               st2T[:, k, :], start=(k == 0), stop=(k == HK - 1))
                    zs = sp.tile([128, HK, TL, B], f32, tag="zsum")
                    ps_u = pp1.tile([128, 2, H], f32, tag="A")
                    for h2 in range(2):
                        a1, a2 = broadcast_tensor_aps(
                            w1ta[:, 2 * h2:2 * h2 + 2, :, :],
                            ps_q2[:, 2 * h2:2 * h2 + 2, None, :])
                        nc.vector.tensor_tensor(
                            zs[:, 2 * h2:2 * h2 + 2, :, :], a1, a2, OP.add)
                        ACT(zt[:, 2 * h2:2 * h2 + 2, :],
                            zs[:, 2 * h2:2 * h2 + 2, :, :], AF.Tanh)
                        for m2 in range(2):
                            for k in (2 * h2, 2 * h2 + 1):
                                MM(ps_u[:, m2, :],
                                   zt[:, k, 128 * m2:128 * m2 + 128],
                                   vtt[:, k, :], start=(k == 0),
                                   stop=(k == HK - 1))
                if not with_q2:
                    ps_u = pp1.tile([128, 2, H], f32, tag="A")
                    for m2 in range(2):
                        for k in range(HK):
                            MM(ps_u[:, m2, :],
                               zt[:, k, 128 * m2:128 * m2 + 128],
                               vtt[:, k, :], start=(k == 0),
                               stop=(k == HK - 1))
                prod = sp.tile([128, 2, H], bf16, tag="prod")
                for m2 in range(2):
                    aw = sp.tile([128, H], bf16, tag="aw")
                    den = sp.tile([128, 1], f32, tag="den")
                    ACT(aw[:, :], ps_u[:, m2, :], AF.Exp,
                        accum_out=den[:, :])
                    rden = sp.tile([128, 1], f32, tag="rden")
                    nc.vector.reciprocal(rden[:, :], den[:, :])
                    nc.vector.scalar_tensor_tensor(
                        prod[:, m2, :], aw[:, :], rden[:, :], encbt[:, m2, :],
                        OP.mult, OP.mult)
                ps_loc = pp1.tile([B, H], f32, tag="C")
                for kt in range(2):
                    MM(ps_loc[:, :], maskl[:, kt, :], prod[:, kt, :],
                       start=(kt == 0), stop=(kt == 1))
                part = sp.tile([B, H], bf16, tag="part")
                nc.vector.tensor_copy(part[:, :], ps_loc[:, :])
                s = j % 2
                nc.gpsimd.dma_start(bnc_in[s].ap(), part[:, :])
                nc.gpsimd.collective_compute(
                    "AllGather", OP.bypass, replica_groups=rg,
                    ins=[bnc_in[s].ap()], outs=[bnc_out[s].ap()])


            def attend_B(j):
                """AG result -> gathered shards into SBUF. saT for the wout of
                step j-1 is built directly: saT = st2T + attns^T, where
                attns^T comes from mask-matmuls on the gathered shards
                (st2T still holds state2 of step j-1 at this point)."""
                s = j % 2
                for q in range(4):
                    nc.sync.dma_start(
                        gath[:, :, 128 * q:128 * q + 128],
                        bnc_out[s][:, :, 128 * q:128 * q + 128])
                if j >= 1:
                    ps_aT = pp1.tile([128, HK, B], f32, tag="small")
                    for m in range(HK):
                        for i2 in range(2):
                            MM(ps_aT[:, m, :],
                               gath[:, i2, 128 * m:128 * m + 128],
                               maskg[:, i2, :], start=(i2 == 0),
                               stop=(i2 == 1))
                    saT = sp.tile([128, HK, B], bf16, tag="saT")
                    sa_tiles[j - 1] = saT
                    nc.vector.tensor_add(saT[:, :, :], st2T[:, :, :],
                                         ps_aT[:, :, :])

            def mm_rz(ps_rz, lhs, w, coff, start, stop):
                for g in range(2):
                    for k in range(HK):
                        MM(ps_rz[:, g, :], lhs[:, k, :],
                           w[:, k, coff + 512 * g:coff + 512 * g + 512],
                           start=(start and k == 0), stop=(stop and k == HK - 1))

            def mm_n(ps_n, sl, lhs, w, coff, start, stop):
                for k in range(HK):
                    MM(ps_n[:, sl, :], lhs[:, k, :], w[:, k, coff:coff + 512],
                       start=(start and k == 0), stop=(stop and k == HK - 1))

            def gru_gh(L, hT, whh):
                """State-side GRU matmuls - independent of the pending AG."""
                rz_tag = "A" if L == 0 else "C"
                ps_rz = pp1.tile([B, 2, H], f32, tag=rz_tag)
                ps_n = pp1.tile([B, 2, H], f32, tag="B")  # 0: i_n, 1: h_n
                mm_rz(ps_rz, hT, whh, 0, True, False)
                mm_n(ps_n, 1, hT, whh, 1024, True, True)
                return ps_rz, ps_n

            def gru_gates(ps_rz, ps_n, h_prev, s_out, on_half=None):
                # two h-halves pipelined across ACT and DVE
                HH = H // 2
                rz_s = sp.tile([B, 2, H], f32, tag="gsig")
                zc = sp.tile([B, H], f32, tag="gzc")
                t1 = sp.tile([B, H], f32, tag="gtmp")
                t2 = sp.tile([B, H], f32, tag="gtmp2")
                p1 = sp.tile([B, H], f32, tag="gp1")
                n_s = sp.tile([B, H], f32, tag="gn")
                p2 = sp.tile([B, H], f32, tag="gp2")
                for h in range(2):
                    hs = slice(HH * h, HH * h + HH)
                    ACT(rz_s[:, 0, hs], ps_rz[:, 0, hs], AF.Sigmoid)
                    nc.vector.tensor_mul(t1[:, hs], rz_s[:, 0, hs],
                                         ps_n[:, 1, hs])
                    ACT(rz_s[:, 1, hs], ps_rz[:, 1, hs], AF.Sigmoid)
                    nc.vector.tensor_add(t2[:, hs], ps_n[:, 0, hs], t1[:, hs])
                    ACT(zc[:, hs], ps_rz[:, 1, hs], AF.Sigmoid, scale=-1.0)
                    nc.vector.tensor_mul(p1[:, hs], rz_s[:, 1, hs],
                                         h_prev[:, hs])
                    ACT(n_s[:, hs], t2[:, hs], AF.Tanh)
                    nc.vector.tensor_mul(p2[:, hs], zc[:, hs], n_s[:, hs])
                    nc.vector.tensor_add(s_out[:, hs], p1[:, hs], p2[:, hs])
                    if on_half is not None:
                        on_half(h)

            def transpose_half(dst, src_t, h):
                ps_T = pp1.tile([128, 2, B], f32, tag="small")
                for i, k in enumerate((2 * h, 2 * h + 1)):
                    TP(ps_T[:, i, :], src_t[:, 128 * k:128 * k + 128],
                       id32[0:B, 0:B])
                ACT(dst[:, 2 * h:2 * h + 2, :], ps_T[:, :, :], AF.Copy)

            def wout_chunks(t, saT, chunks):
                for c in chunks:
                    ps_o = pp2.tile([B, 500], f32, tag="out")
                    for k in range(HK):
                        MM(ps_o[:, :], saT[:, k, :],
                           wout[:, k, 500 * c:500 * c + 500],
                           start=(k == 0), stop=(k == HK - 1))
                    o_sb = sp.tile([B, 500], bf16, tag="osb")
                    if c % 2 == 0:
                        nc.vector.tensor_copy(o_sb[:, :], ps_o[:, :])
                    else:
                        ACT(o_sb[:, :], ps_o[:, :], AF.Copy)
                    E[c % 2].dma_start(
                        d_out[32 * t:32 * t + 32, 500 * c:500 * c + 500],
                        o_sb[:, :])

            # ---------- initial attention (attns0) ----------
            attend_A(0, with_q2=False)
            attend_B(0)

            # ---------- decode steps ----------
            # gh1 of step 0 (zero state): filler emitted before attend_B(0)
            g1 = gru_gh(0, st2T, whh1)
            attend_B(0)
            saT_prev = None
            for t in range(TDEC):
                # x^T directly from the gathered shards:
                #   xT[h,b] = sum_r gath[r,h] * WrR^T[r%32,b] + Xl[b,t]
                ps_xt = pp1.tile([128, HK, B], f32, tag="small")
                for m in range(HK):
                    for i2 in range(2):
                        MM(ps_xt[:, m, :], gath[:, i2, 128 * m:128 * m + 128],
                           rhsg[:, i2, :], start=(i2 == 0), stop=False)
                    MM(ps_xt[:, m, :], xt_lhs[32:33, 128 * m:128 * m + 128],
                       rhsx[32:33, t, :], start=False, stop=True)
                nc.vector.tensor_copy(xt_sb[:, :, :], ps_xt[:, :, :])

                s1 = sp.tile([B, H], f32, tag="st1")
                mm_rz(g1[0], xt_sb, wih, 0, False, True)
                mm_n(g1[1], 0, xt_sb, wih, 1024, True, True)
                # layer-2 x-side matmuls (fill the gates1 window)
                ps2_rz = pp1.tile([B, 2, H], f32, tag="C")
                ps2_n = pp1.tile([B, 2, H], f32, tag="B")
                mm_rz(ps2_rz, xt_sb, wih, 1536, True, False)
                mm_n(ps2_n, 0, xt_sb, wih, 1536 + 1024, True, True)

                def after1(h):
                    # as each h-half of state1 lands: transpose it and start
                    # the h-side layer-2 matmuls for those k-tiles
                    transpose_half(st1T, s1, h)
                    for g in range(2):
                        for k in (2 * h, 2 * h + 1):
                            MM(ps2_rz[:, g, :], st1T[:, k, :],
                               whh2[:, k, 512 * g:512 * g + 512],
                               start=False, stop=(k == HK - 1))
                    for k in (2 * h, 2 * h + 1):
                        MM(ps2_n[:, 1, :], st1T[:, k, :], whh2[:, k, 1024:1536],
                           start=(k == 0), stop=(k == HK - 1))

                gru_gates(g1[0], g1[1], state, s1, on_half=after1)

                if t >= 1:
                    saT_prev = sa_tiles.pop(t - 1)
                    wout_chunks(t - 1, saT_prev, [0, 1, 2, 3])

                def after2(h):
                    transpose_half(st2T, state, h)

                gru_gates(ps2_rz, ps2_n, s1, state, on_half=after2)

                attend_A(t + 1, with_q2=True)
                # AG in flight: next step's gh1 + second half of wout t-1
                if t < TDEC - 1:
                    g1 = gru_gh(0, st2T, whh1)
                attend_B(t + 1)

            saT = sa_tiles.pop(TDEC - 1)
            wout_chunks(TDEC - 1, saT, [0, 1, 2, 3])
            nc.sync.dma_start(d_state.ap(), state[:, :])

    nc.compile()
    return nc


def _get_built():
    global _BUILT
    if _BUILT is None:
        _BUILT = _build()
    return _BUILT


def _prep(inputs):
    import ml_dtypes
    bf = ml_dtypes.bfloat16

    def f(x):
        return np.asarray(x, np.float32)

    enc = f(inputs["encoder_output"])            # [B, T, H]
    dec = f(inputs["decoder_input"])             # [B, TDEC]
    W1, W2, vT = f(inputs["W1"]), f(inputs["W2"]), f(inputs["vT"])
    Wr, br = f(inputs["Wr"]), f(inputs["br"])
    Wih1, Whh1 = f(inputs["Wih1"]), f(inputs["Whh1"])
    Wih2, Whh2 = f(inputs["Wih2"]), f(inputs["Whh2"])
    Wout = f(inputs["Wout"])

    def sb_layout(M):  # [512, X] -> [128, 4, X]
        X = M.shape[1]
        return np.ascontiguousarray(
            M.reshape(HK, 128, X).transpose(1, 0, 2)).astype(bf)

    w12t = sb_layout((W1 + W2).T)
    w1t = sb_layout(W1.T)
    vtt = sb_layout(vT.T)
    w2t = sb_layout(W2.T)
    wihT = sb_layout(np.concatenate([Wih1.T, Wih2.T], axis=1))
    whh1T = sb_layout(Whh1.T)
    whh2T = sb_layout(Whh2.T)

    # rhs for x^T matmul: rows 0-31 Wr_r^T, row 32 = Xl + br
    Xl = Wr[:, :B] @ dec + br[:, None]           # [B, TDEC]
    rhsx = np.zeros((33, TDEC, B), np.float32)
    rhsx[:B] = np.broadcast_to(Wr[:, B:].T[:, None, :], (B, TDEC, B))
    rhsx[B] = Xl.T
    rhsx = rhsx.astype(bf)

    # masks
    maskl = np.zeros((128, 2, B), np.float32)
    for kt in range(2):
        for k in range(128):
            maskl[k, kt, k % B] = 1.0
    maskg = np.zeros((128, 2, B), np.float32)
    rhsg = np.zeros((128, 2, B), np.float32)
    WrRT = Wr[:, B:].T                           # [k, b]
    for i2 in range(2):
        for k in range(128):
            maskg[k, i2, (2 * k + i2) % B] = 1.0
            rhsg[k, i2, :] = WrRT[(2 * k + i2) % B, :]
    maskl = maskl.astype(bf)
    maskg = maskg.astype(bf)
    rhsg = rhsg.astype(bf)
    id32 = np.eye(128, dtype=np.float32)
    id16 = np.eye(128, dtype=np.float32).astype(bf)

    shared = dict(w12t=w12t, w1t=w1t, vtt=vtt, w2t=w2t, wih=wihT,
                  whh1=whh1T, whh2=whh2T, rhsx=rhsx, maskl=maskl,
                  maskg=maskg, rhsg=rhsg, id32=id32, id16=id16)

    in_maps = []
    for c in range(NCORES):
        enc_sh = enc[:, TL * c:TL * c + TL, :]               # [B, TL, H]
        # t-major local rows: bt = tl*B + b
        encT = sb_layout(enc_sh.transpose(2, 1, 0).reshape(H, BT))
        encbt = np.ascontiguousarray(
            enc_sh.transpose(1, 0, 2).reshape(BT, H)
            .reshape(2, 128, H).transpose(1, 0, 2)
        ).astype(bf)
        woutT = sb_layout(Wout.T[:, VS * c:VS * c + VS])
        m = dict(shared)
        m.update(encT=encT, encbt=encbt, wout=woutT)
        in_maps.append(m)
    return in_maps


def _run(inputs, trace=False, **kw):
    from concourse import bass_utils
    nc = _get_built()
    in_maps = _prep(inputs)
    res = bass_utils.run_bass_kernel_spmd(
        nc, in_maps, core_ids=list(range(NCORES)), trace=trace, **kw)
    outs = [np.asarray(r["out"], np.float32).reshape(TDEC, B, VS)
            for r in res.results]
    out = np.concatenate(outs, axis=2)
    out = out + np.asarray(inputs["bout"], np.float32)[None, None, :]
    st = np.asarray(res.results[0]["state_out"], np.float32)
    return (out, st), res


def kernel(**inputs):
    (out, st), _ = _run(inputs)
    return out, st


# revision 21
# speedup vs baseline: 1.0059x; 1.0059x over previous
# Distributed Trainium2 kernel for nn_DecoderRNN (attention decoder RNN).
#
# Sharding: encoder T axis sharded 8-way for the per-step attention
# (partials AllGathered each step and reduced on-chip with a ones-matmul);
# GRU replicated on all cores; Wout sharded over vocab (2000/core).
# All matmuls run in bf16 with f32 PSUM accumulation. Activations that feed
# contractions over H are produced directly in transposed [H, B] layout to
# avoid per-step transposes where possible.
#
# Biases: br is folded into the precomputed per-step rhs (exact); bout is
# added host-side; bih*/bhh* are zeros by the problem spec (fill: zeros).

import numpy as np

NCORES = 8
B, T, TDEC, H, V = 32, 64, 48, 512, 16000
TL = T // NCORES          # 8 encoder timesteps per core
BT = B * TL               # 256 local (b, t) rows
VS = V // NCORES          # 2000 vocab entries per core
HK = H // 128             # 4 contraction k-tiles

_BUILT = None


def _build():
    from concourse import bacc, mybir
    from concourse import tile
    from concourse.bass import broadcast_tensor_aps

    f32 = mybir.dt.float32
    bf16 = mybir.dt.bfloat16
    AX = mybir.AxisListType
    OP = mybir.AluOpType
    AF = mybir.ActivationFunctionType

    nc = bacc.Bacc("TRN2", target_bir_lowering=False, num_devices=NCORES)

    dram_in = lambda name, shape, dt: nc.dram_tensor(name, shape, dt, kind="ExternalInput")
    # ---- external inputs (per-core shards prepared host-side) ----
    d_encT = dram_in("encT", [128, HK, BT], bf16)      # enc^T shard  [h, (b,tl)]
    d_encbt = dram_in("encbt", [128, 2, H], bf16)      # enc shard    [(b,tl), h]
    d_w12t = dram_in("w12t", [128, HK, H], bf16)       # (W1+W2)^T
    d_w1t = dram_in("w1t", [128, HK, H], bf16)         # W1^T
    d_vtt = dram_in("vtt", [128, HK, H], bf16)         # vT^T
    d_w2t = dram_in("w2t", [128, HK, H], bf16)         # W2^T
    d_wih = dram_in("wih", [128, HK, 6 * H], bf16)     # [Wih1^T | Wih2^T]
    d_whh1 = dram_in("whh1", [128, HK, 3 * H], bf16)   # Whh1^T
    d_whh2 = dram_in("whh2", [128, HK, 3 * H], bf16)   # Whh2^T
    d_wout = dram_in("wout", [128, HK, VS], bf16)      # Wout^T vocab shard
    d_rhsx = dram_in("rhsx", [33, TDEC, B], bf16)      # [Wr_r^T ; Xl_t + br]
    d_maskl = dram_in("maskl", [128, 2, B], bf16)      # local t-sum masks
    d_maskg = dram_in("maskg", [128, 2, B], bf16)      # gathered core-sum masks
    d_rhsg = dram_in("rhsg", [128, 2, B], bf16)        # WrR^T expanded to gathered rows
    d_id32 = dram_in("id32", [128, 128], f32)
    d_id16 = dram_in("id16", [128, 128], bf16)

    d_out = nc.dram_tensor("out", [TDEC * B, VS], bf16, kind="ExternalOutput")
    d_state = nc.dram_tensor("state_out", [B, H], f32, kind="ExternalOutput")

    # ---- internal DRAM (collective bounce ring) ----
    bnc_in = [nc.dram_tensor(f"agin{i}", [B, H], bf16) for i in range(2)]
    bnc_out = [
        nc.dram_tensor(f"agout{i}", [128, 2, H], bf16, addr_space="Shared")
        for i in range(2)
    ]

    # ---- persistent SBUF ----
    sb = nc.alloc_sbuf_tensor
    encT = sb("s_encT", [128, HK, BT], bf16)
    encbt = sb("s_encbt", [128, 2, H], bf16)
    w12t = sb("s_w12t", [128, HK, H], bf16)
    w1tw = sb("s_w1tw", [128, HK, H], bf16)
    vtt = sb("s_vtt", [128, HK, H], bf16)
    w2t = sb("s_w2t", [128, HK, H], bf16)
    wih = sb("s_wih", [128, HK, 6 * H], bf16)
    whh1 = sb("s_whh1", [128, HK, 3 * H], bf16)
    whh2 = sb("s_whh2", [128, HK, 3 * H], bf16)
    wout = sb("s_wout", [128, HK, VS], bf16)
    rhsx = sb("s_rhsx", [33, TDEC, B], bf16)
    maskl = sb("s_maskl", [128, 2, B], bf16)
    maskg = sb("s_maskg", [128, 2, B], bf16)
    rhsg = sb("s_rhsg", [128, 2, B], bf16)
    id32 = sb("s_id32", [128, 128], f32)
    id16 = sb("s_id16", [128, 128], bf16)
    w1ta = sb("s_w1ta", [128, HK, TL, B], f32)   # w1^T acts, t-major free dims
    xt_lhs = sb("s_xtlhs", [33, H], bf16)        # rows 0-31: attns, row 32: ones
    xt_sb = sb("s_xt", [128, HK, B], bf16)       # x^T
    st1T = sb("s_st1T", [128, HK, B], bf16)      # state1^T
    st2T = sb("s_st2T", [128, HK, B], bf16)      # state2^T (carry)
    state = sb("s_state", [B, H], f32)           # carry state (normal layout)
    zt = sb("s_zt", [128, HK, BT], bf16)         # tanh(w1+q2)^T
    gath = sb("s_gath", [128, 2, H], bf16)       # gathered partials

    MM = nc.tensor.matmul
    TP = nc.tensor.transpose
    ACT = nc.scalar.activation
    rg = [list(range(NCORES))]

    with tile.TileContext(nc) as tc:
        with (
            tc.tile_pool(name="sp", bufs=2) as sp,
            tc.tile_pool(name="pp1", bufs=1, space="PSUM") as pp1,
            tc.tile_pool(name="pp2", bufs=1, space="PSUM") as pp2,
        ):
            # ---------- preload ----------
            # engines used purely as DMA triggers
            E = [nc.sync, nc.scalar, nc.gpsimd, nc.sync]
            loads = [
                (encT, d_encT), (w12t, d_w12t), (w1tw, d_w1t), (encbt, d_encbt),
                (vtt, d_vtt), (w2t, d_w2t), (rhsx, d_rhsx), (maskl, d_maskl),
                (maskg, d_maskg), (id32, d_id32), (id16, d_id16), (rhsg, d_rhsg),
            ]
            for i, (dst, src) in enumerate(loads):
                E[i % 4].dma_start(dst.ap(), src.ap())
            # big weights, chunked across engines
            for k in range(HK):
                nc.sync.dma_start(wih[:, k, :], d_wih[:, k, :])
                E[k % 4].dma_start(whh1[:, k, :], d_whh1[:, k, :])
                E[(k + 1) % 4].dma_start(whh2[:, k, :], d_whh2[:, k, :])
                nc.scalar.dma_start(wout[:, k, :], d_wout[:, k, :])

            nc.gpsimd.memset(xt_lhs[32:33, :], 1.0)
            nc.gpsimd.memset(state[:, :], 0.0)
            nc.gpsimd.memset(st2T[:, :, :], 0.0)

            # ---------- precompute w1^T acts and z0 ----------
            for m in range(HK):
                ps_w = pp1.tile([128, BT], f32, tag="A")
                for k in range(HK):
                    MM(ps_w[:, :], w1tw[:, k, 128 * m:128 * m + 128], encT[:, k, :],
                       start=(k == 0), stop=(k == HK - 1))
                ACT(w1ta[:, m, :, :], ps_w[:, :], AF.Copy)
            for m in range(HK):
                ps_z = pp1.tile([128, BT], f32, tag="C")
                for k in range(HK):
                    MM(ps_z[:, :], w12t[:, k, 128 * m:128 * m + 128], encT[:, k, :],
                       start=(k == 0), stop=(k == HK - 1))
                ACT(zt[:, m, :], ps_z[:, :], AF.Tanh)

            sa_tiles = {}

            def attend_A(j, with_q2):
                """q2^T -> z^T -> u -> softmax -> local partial -> AG trigger."""
                if with_q2:
                    ps_q2 = pp1.tile([128, HK, B], f32, tag="small")
                    for m in range(HK):
                        for k in range(HK):
                            MM(ps_q2[:, m, :], w2t[:, k, 128 * m:128 * m + 128],
                               st2T[:, k, :], start=(k == 0), stop=(k == HK - 1))
                    zs = sp.tile([128, HK, TL, B], f32, tag="zsum")
                    ps_u = pp1.tile([128, 2, H], f32, tag="A")
                    for h2 in range(2):
                        a1, a2 = broadcast_tensor_aps(
                            w1ta[:, 2 * h2:2 * h2 + 2, :, :],
                            ps_q2[:, 2 * h2:2 * h2 + 2, None, :])
                        nc.vector.tensor_tensor(
                            zs[:, 2 * h2:2 * h2 + 2, :, :], a1, a2, OP.add)
                        ACT(zt[:, 2 * h2:2 * h2 + 2, :],
                            zs[:, 2 * h2:2 * h2 + 2, :, :], AF.Tanh)
                        for m2 in range(2):
                            for k in (2 * h2, 2 * h2 + 1):
                                MM(ps_u[:, m2, :],
                                   zt[:, k, 128 * m2:128 * m2 + 128],
                                   vtt[:, k, :], start=(k == 0),
                                   stop=(k == HK - 1))
                if not with_q2:
                    ps_u = pp1.tile([128, 2, H], f32, tag="A")
                    for m2 in range(2):
                        for k in range(HK):
                            MM(ps_u[:, m2, :],
                               zt[:, k, 128 * m2:128 * m2 + 128],
                               vtt[:, k, :], start=(k == 0),
                               stop=(k == HK - 1))
                prod = sp.tile([128, 2, H], bf16, tag="prod")
                for m2 in range(2):
                    aw = sp.tile([128, H], bf16, tag="aw")
                    den = sp.tile([128, 1], f32, tag="den")
                    ACT(aw[:, :], ps_u[:, m2, :], AF.Exp,
                        accum_out=den[:, :])
                    rden = sp.tile([128, 1], f32, tag="rden")
                    nc.vector.reciprocal(rden[:, :], den[:, :])
                    nc.vector.scalar_tensor_tensor(
                        prod[:, m2, :], aw[:, :], rden[:, :], encbt[:, m2, :],
                        OP.mult, OP.mult)
                ps_loc = pp1.tile([B, H], f32, tag="C")
                for kt in range(2):
                    MM(ps_loc[:, :], maskl[:, kt, :], prod[:, kt, :],
                       start=(kt == 0), stop=(kt == 1))
                part = sp.tile([B, H], bf16, tag="part")
                ACT(part[:, :], ps_loc[:, :], AF.Copy)
                s = j % 2
                nc.sync.dma_start(bnc_in[s].ap(), part[:, :])
                nc.gpsimd.collective_compute(
                    "AllGather", OP.bypass, replica_groups=rg,
                    ins=[bnc_in[s].ap()], outs=[bnc_out[s].ap()])


            def attend_B(j):
                """AG result -> gathered shards into SBUF. saT for the wout of
                step j-1 is built directly: saT = st2T + attns^T, where
                attns^T comes from mask-matmuls on the gathered shards
                (st2T still holds state2 of step j-1 at this point)."""
                s = j % 2
                for q in range(4):
                    nc.sync.dma_start(
                        gath[:, :, 128 * q:128 * q + 128],
                        bnc_out[s][:, :, 128 * q:128 * q + 128])
                if j >= 1:
                    ps_aT = pp1.tile([128, HK, B], f32, tag="small")
                    for m in range(HK):
                        for i2 in range(2):
                            MM(ps_aT[:, m, :],
                               gath[:, i2, 128 * m:128 * m + 128],
                               maskg[:, i2, :], start=(i2 == 0),
                               stop=(i2 == 1))
                    saT = sp.tile([128, HK, B], bf16, tag="saT")
                    sa_tiles[j - 1] = saT
                    nc.vector.tensor_add(saT[:, :, :], st2T[:, :, :],
                                         ps_aT[:, :, :])

            def mm_rz(ps_rz, lhs, w, coff, start, stop):
                for g in range(2):
                    for k in range(HK):
                        MM(ps_rz[:, g, :], lhs[:, k, :],
                           w[:, k, coff + 512 * g:coff + 512 * g + 512],
                           start=(start and k == 0), stop=(stop and k == HK - 1))

            def mm_n(ps_n, sl, lhs, w, coff, start, stop):
                for k in range(HK):
                    MM(ps_n[:, sl, :], lhs[:, k, :], w[:, k, coff:coff + 512],
                       start=(start and k == 0), stop=(stop and k == HK - 1))

            def gru_gh(L, hT, whh):
                """State-side GRU matmuls - independent of the pending AG."""
                rz_tag = "A" if L == 0 else "C"
                ps_rz = pp1.tile([B, 2, H], f32, tag=rz_tag)
                ps_n = pp1.tile([B, 2, H], f32, tag="B")  # 0: i_n, 1: h_n
                mm_rz(ps_rz, hT, whh, 0, True, False)
                mm_n(ps_n, 1, hT, whh, 1024, True, True)
                return ps_rz, ps_n

            def gru_gates(ps_rz, ps_n, h_prev, s_out, on_half=None):
                # two h-halves pipelined across ACT and DVE
                HH = H // 2
                rz_s = sp.tile([B, 2, H], f32, tag="gsig")
                zc = sp.tile([B, H], f32, tag="gzc")
                t1 = sp.tile([B, H], f32, tag="gtmp")
                t2 = sp.tile([B, H], f32, tag="gtmp2")
                p1 = sp.tile([B, H], f32, tag="gp1")
                n_s = sp.tile([B, H], f32, tag="gn")
                p2 = sp.tile([B, H], f32, tag="gp2")
                for h in range(2):
                    hs = slice(HH * h, HH * h + HH)
                    ACT(rz_s[:, 0, hs], ps_rz[:, 0, hs], AF.Sigmoid)
                    nc.vector.tensor_mul(t1[:, hs], rz_s[:, 0, hs],
                                         ps_n[:, 1, hs])
                    ACT(rz_s[:, 1, hs], ps_rz[:, 1, hs], AF.Sigmoid)
                    nc.vector.tensor_add(t2[:, hs], ps_n[:, 0, hs], t1[:, hs])
                    ACT(zc[:, hs], ps_rz[:, 1, hs], AF.Sigmoid, scale=-1.0)
                    nc.vector.tensor_mul(p1[:, hs], rz_s[:, 1, hs],
                                         h_prev[:, hs])
                    ACT(n_s[:, hs], t2[:, hs], AF.Tanh)
                    nc.vector.tensor_mul(p2[:, hs], zc[:, hs], n_s[:, hs])
                    nc.vector.tensor_add(s_out[:, hs], p1[:, hs], p2[:, hs])
                    if on_half is not None:
                        on_half(h)

            def transpose_half(dst, src_t, h):
                ps_T = pp1.tile([128, 2, B], f32, tag="small")
                for i, k in enumerate((2 * h, 2 * h + 1)):
                    TP(ps_T[:, i, :], src_t[:, 128 * k:128 * k + 128],
                       id32[0:B, 0:B])
                ACT(dst[:, 2 * h:2 * h + 2, :], ps_T[:, :, :], AF.Copy)

            def wout_chunks(t, saT, chunks):
                for c in chunks:
                    ps_o = pp2.tile([B, 500], f32, tag="out")
                    for k in range(HK):
                        MM(ps_o[:, :], saT[:, k, :],
                           wout[:, k, 500 * c:500 * c + 500],
                           start=(k == 0), stop=(k == HK - 1))
                    o_sb = sp.tile([B, 500], bf16, tag="osb")
                    if c % 2 == 0:
                        nc.vector.tensor_copy(o_sb[:, :], ps_o[:, :])
                    else:
                        ACT(o_sb[:, :], ps_o[:, :], AF.Copy)
                    nc.scalar.dma_start(
                        d_out[32 * t:32 * t + 32, 500 * c:500 * c + 500],
                        o_sb[:, :])

            # ---------- initial attention (attns0) ----------
            attend_A(0, with_q2=False)
            attend_B(0)

            # ---------- decode steps ----------
            # gh1 of step 0 (zero state): filler emitted before attend_B(0)
            g1 = gru_gh(0, st2T, whh1)
            attend_B(0)
            saT_prev = None
            for t in range(TDEC):
                # x^T directly from the gathered shards:
                #   xT[h,b] = sum_r gath[r,h] * WrR^T[r%32,b] + Xl[b,t]
                ps_xt = pp1.tile([128, HK, B], f32, tag="small")
                for m in range(HK):
                    for i2 in range(2):
                        MM(ps_xt[:, m, :], gath[:, i2, 128 * m:128 * m + 128],
                           rhsg[:, i2, :], start=(i2 == 0), stop=False)
                    MM(ps_xt[:, m, :], xt_lhs[32:33, 128 * m:128 * m + 128],
                       rhsx[32:33, t, :], start=False, stop=True)
                for m in range(HK):
                    nc.vector.tensor_copy(xt_sb[:, m, :], ps_xt[:, m, :])

                s1 = sp.tile([B, H], f32, tag="st1")
                mm_rz(g1[0], xt_sb, wih, 0, False, True)
                mm_n(g1[1], 0, xt_sb, wih, 1024, True, True)
                # layer-2 x-side matmuls (fill the gates1 window)
                ps2_rz = pp1.tile([B, 2, H], f32, tag="C")
                ps2_n = pp1.tile([B, 2, H], f32, tag="B")
                mm_rz(ps2_rz, xt_sb, wih, 1536, True, False)
                mm_n(ps2_n, 0, xt_sb, wih, 1536 + 1024, True, True)

                def after1(h):
                    # as each h-half of state1 lands: transpose it and start
                    # the h-side layer-2 matmuls for those k-tiles
                    transpose_half(st1T, s1, h)
                    for g in range(2):
                        for k in (2 * h, 2 * h + 1):
                            MM(ps2_rz[:, g, :], st1T[:, k, :],
                               whh2[:, k, 512 * g:512 * g + 512],
                               start=False, stop=(k == HK - 1))
                    for k in (2 * h, 2 * h + 1):
                        MM(ps2_n[:, 1, :], st1T[:, k, :], whh2[:, k, 1024:1536],
                           start=(k == 0), stop=(k == HK - 1))

                gru_gates(g1[0], g1[1], state, s1, on_half=after1)

                if t >= 1:
                    saT_prev = sa_tiles.pop(t - 1)
                    wout_chunks(t - 1, saT_prev, [0, 1, 2, 3])

                def after2(h):
                    transpose_half(st2T, state, h)

                gru_gates(ps2_rz, ps2_n, s1, state, on_half=after2)

                attend_A(t + 1, with_q2=True)
                # AG in flight: next step's gh1 + second half of wout t-1
                if t < TDEC - 1:
                    g1 = gru_gh(0, st2T, whh1)
                attend_B(t + 1)

            saT = sa_tiles.pop(TDEC - 1)
            wout_chunks(TDEC - 1, saT, [0, 1, 2, 3])
            nc.sync.dma_start(d_state.ap(), state[:, :])

    nc.compile()
    return nc


def _get_built():
    global _BUILT
    if _BUILT is None:
        _BUILT = _build()
    return _BUILT


def _prep(inputs):
    import ml_dtypes
    bf = ml_dtypes.bfloat16

    def f(x):
        return np.asarray(x, np.float32)

    enc = f(inputs["encoder_output"])            # [B, T, H]
    dec = f(inputs["decoder_input"])             # [B, TDEC]
    W1, W2, vT = f(inputs["W1"]), f(inputs["W2"]), f(inputs["vT"])
    Wr, br = f(inputs["Wr"]), f(inputs["br"])
    Wih1, Whh1 = f(inputs["Wih1"]), f(inputs["Whh1"])
    Wih2, Whh2 = f(inputs["Wih2"]), f(inputs["Whh2"])
    Wout = f(inputs["Wout"])

    def sb_layout(M):  # [512, X] -> [128, 4, X]
        X = M.shape[1]
        return np.ascontiguousarray(
            M.reshape(HK, 128, X).transpose(1, 0, 2)).astype(bf)

    w12t = sb_layout((W1 + W2).T)
    w1t = sb_layout(W1.T)
    vtt = sb_layout(vT.T)
    w2t = sb_layout(W2.T)
    wihT = sb_layout(np.concatenate([Wih1.T, Wih2.T], axis=1))
    whh1T = sb_layout(Whh1.T)
    whh2T = sb_layout(Whh2.T)

    # rhs for x^T matmul: rows 0-31 Wr_r^T, row 32 = Xl + br
    Xl = Wr[:, :B] @ dec + br[:, None]           # [B, TDEC]
    rhsx = np.zeros((33, TDEC, B), np.float32)
    rhsx[:B] = np.broadcast_to(Wr[:, B:].T[:, None, :], (B, TDEC, B))
    rhsx[B] = Xl.T
    rhsx = rhsx.astype(bf)

    # masks
    maskl = np.zeros((128, 2, B), np.float32)
    for kt in range(2):
        for k in range(128):
            maskl[k, kt, k % B] = 1.0
    maskg = np.zeros((128, 2, B), np.float32)
    rhsg = np.zeros((128, 2, B), np.float32)
    WrRT = Wr[:, B:].T                           # [k, b]
    for i2 in range(2):
        for k in range(128):
            maskg[k, i2, (2 * k + i2) % B] = 1.0
            rhsg[k, i2, :] = WrRT[(2 * k + i2) % B, :]
    maskl = maskl.astype(bf)
    maskg = maskg.astype(bf)
    rhsg = rhsg.astype(bf)
    id32 = np.eye(128, dtype=np.float32)
    id16 = np.eye(128, dtype=np.float32).astype(bf)

    shared = dict(w12t=w12t, w1t=w1t, vtt=vtt, w2t=w2t, wih=wihT,
                  whh1=whh1T, whh2=whh2T, rhsx=rhsx, maskl=maskl,
                  maskg=maskg, rhsg=rhsg, id32=id32, id16=id16)

    in_maps = []
    for c in range(NCORES):
        enc_sh = enc[:, TL * c:TL * c + TL, :]               # [B, TL, H]
        # t-major local rows: bt = tl*B + b
        encT = sb_layout(enc_sh.transpose(2, 1, 0).reshape(H, BT))
        encbt = np.ascontiguousarray(
            enc_sh.transpose(1, 0, 2).reshape(BT, H)
            .reshape(2, 128, H).transpose(1, 0, 2)
        ).astype(bf)
        woutT = sb_layout(Wout.T[:, VS * c:VS * c + VS])
        m = dict(shared)
        m.update(encT=encT, encbt=encbt, wout=woutT)
        in_maps.append(m)
    return in_maps


def _run(inputs, trace=False, **kw):
    from concourse import bass_utils
    nc = _get_built()
    in_maps = _prep(inputs)
    res = bass_utils.run_bass_kernel_spmd(
        nc, in_maps, core_ids=list(range(NCORES)), trace=trace, **kw)
    outs = [np.asarray(r["out"], np.float32).reshape(TDEC, B, VS)
            for r in res.results]
    out = np.concatenate(outs, axis=2)
    out = out + np.asarray(inputs["bout"], np.float32)[None, None, :]
    st = np.asarray(res.results[0]["state_out"], np.float32)
    return (out, st), res


def kernel(**inputs):
    (out, st), _ = _run(inputs)
    return out, st


# revision 22
# speedup vs baseline: 1.0192x; 1.0132x over previous
# Distributed Trainium2 kernel for nn_DecoderRNN (attention decoder RNN).
#
# Sharding: encoder T axis sharded 8-way for the per-step attention
# (partials AllGathered each step and reduced on-chip with a ones-matmul);
# GRU replicated on all cores; Wout sharded over vocab (2000/core).
# All matmuls run in bf16 with f32 PSUM accumulation. Activations that feed
# contractions over H are produced directly in transposed [H, B] layout to
# avoid per-step transposes where possible.
#
# Biases: br is folded into the precomputed per-step rhs (exact); bout is
# added host-side; bih*/bhh* are zeros by the problem spec (fill: zeros).

import numpy as np

NCORES = 8
B, T, TDEC, H, V = 32, 64, 48, 512, 16000
TL = T // NCORES          # 8 encoder timesteps per core
BT = B * TL               # 256 local (b, t) rows
VS = V // NCORES          # 2000 vocab entries per core
HK = H // 128             # 4 contraction k-tiles

_BUILT = None


def _build():
    from concourse import bacc, mybir
    from concourse import tile
    from concourse.bass import broadcast_tensor_aps

    f32 = mybir.dt.float32
    bf16 = mybir.dt.bfloat16
    AX = mybir.AxisListType
    OP = mybir.AluOpType
    AF = mybir.ActivationFunctionType

    nc = bacc.Bacc("TRN2", target_bir_lowering=False, num_devices=NCORES)

    dram_in = lambda name, shape, dt: nc.dram_tensor(name, shape, dt, kind="ExternalInput")
    # ---- external inputs (per-core shards prepared host-side) ----
    d_encT = dram_in("encT", [128, HK, BT], bf16)      # enc^T shard  [h, (b,tl)]
    d_encbt = dram_in("encbt", [128, 2, H], bf16)      # enc shard    [(b,tl), h]
    d_w12t = dram_in("w12t", [128, HK, H], bf16)       # (W1+W2)^T
    d_w1t = dram_in("w1t", [128, HK, H], bf16)         # W1^T
    d_vtt = dram_in("vtt", [128, HK, H], bf16)         # vT^T
    d_w2t = dram_in("w2t", [128, HK, H], bf16)         # W2^T
    d_wih = dram_in("wih", [128, HK, 6 * H], bf16)     # [Wih1^T | Wih2^T]
    d_whh1 = dram_in("whh1", [128, HK, 3 * H], bf16)   # Whh1^T
    d_whh2 = dram_in("whh2", [128, HK, 3 * H], bf16)   # Whh2^T
    d_wout = dram_in("wout", [128, HK, VS], bf16)      # Wout^T vocab shard
    d_rhsx = dram_in("rhsx", [33, TDEC, B], bf16)      # [Wr_r^T ; Xl_t + br]
    d_maskl = dram_in("maskl", [128, 2, B], bf16)      # local t-sum masks
    d_maskg = dram_in("maskg", [128, 2, B], bf16)      # gathered core-sum masks
    d_rhsg = dram_in("rhsg", [128, 2, B], bf16)        # WrR^T expanded to gathered rows
    d_id32 = dram_in("id32", [128, 128], f32)
    d_id16 = dram_in("id16", [128, 128], bf16)

    d_out = nc.dram_tensor("out", [TDEC * B, VS], bf16, kind="ExternalOutput")
    d_state = nc.dram_tensor("state_out", [B, H], f32, kind="ExternalOutput")

    # ---- internal DRAM (collective bounce ring) ----
    bnc_in = [nc.dram_tensor(f"agin{i}", [B, H], bf16) for i in range(2)]
    bnc_out = [
        nc.dram_tensor(f"agout{i}", [128, 2, H], bf16, addr_space="Shared")
        for i in range(2)
    ]

    # ---- persistent SBUF ----
    sb = nc.alloc_sbuf_tensor
    encT = sb("s_encT", [128, HK, BT], bf16)
    encbt = sb("s_encbt", [128, 2, H], bf16)
    w12t = sb("s_w12t", [128, HK, H], bf16)
    w1tw = sb("s_w1tw", [128, HK, H], bf16)
    vtt = sb("s_vtt", [128, HK, H], bf16)
    w2t = sb("s_w2t", [128, HK, H], bf16)
    wih = sb("s_wih", [128, HK, 6 * H], bf16)
    whh1 = sb("s_whh1", [128, HK, 3 * H], bf16)
    whh2 = sb("s_whh2", [128, HK, 3 * H], bf16)
    wout = sb("s_wout", [128, HK, VS], bf16)
    rhsx = sb("s_rhsx", [33, TDEC, B], bf16)
    maskl = sb("s_maskl", [128, 2, B], bf16)
    maskg = sb("s_maskg", [128, 2, B], bf16)
    rhsg = sb("s_rhsg", [128, 2, B], bf16)
    id32 = sb("s_id32", [128, 128], f32)
    id16 = sb("s_id16", [128, 128], bf16)
    w1ta = sb("s_w1ta", [128, HK, TL, B], f32)   # w1^T acts, t-major free dims
    xt_lhs = sb("s_xtlhs", [33, H], bf16)        # rows 0-31: attns, row 32: ones
    xt_sb = sb("s_xt", [128, HK, B], bf16)       # x^T
    st1T = sb("s_st1T", [128, HK, B], bf16)      # state1^T
    st2T = sb("s_st2T", [128, HK, B], bf16)      # state2^T (carry)
    state = sb("s_state", [B, H], f32)           # carry state (normal layout)
    zt = sb("s_zt", [128, HK, BT], bf16)         # tanh(w1+q2)^T
    gath = sb("s_gath", [128, 2, H], bf16)       # gathered partials

    MM = nc.tensor.matmul
    TP = nc.tensor.transpose
    ACT = nc.scalar.activation
    rg = [list(range(NCORES))]

    with tile.TileContext(nc) as tc:
        with (
            tc.tile_pool(name="sp", bufs=2) as sp,
            tc.tile_pool(name="pp1", bufs=1, space="PSUM") as pp1,
            tc.tile_pool(name="pp2", bufs=1, space="PSUM") as pp2,
        ):
            # ---------- preload ----------
            # engines used purely as DMA triggers
            E = [nc.sync, nc.scalar, nc.gpsimd, nc.sync]
            loads = [
                (encT, d_encT), (w12t, d_w12t), (w1tw, d_w1t), (encbt, d_encbt),
                (vtt, d_vtt), (w2t, d_w2t), (rhsx, d_rhsx), (maskl, d_maskl),
                (maskg, d_maskg), (id32, d_id32), (id16, d_id16), (rhsg, d_rhsg),
            ]
            for i, (dst, src) in enumerate(loads):
                E[i % 4].dma_start(dst.ap(), src.ap())
            # big weights, chunked across engines
            for k in range(HK):
                nc.sync.dma_start(wih[:, k, :], d_wih[:, k, :])
                E[k % 4].dma_start(whh1[:, k, :], d_whh1[:, k, :])
                E[(k + 1) % 4].dma_start(whh2[:, k, :], d_whh2[:, k, :])
                nc.scalar.dma_start(wout[:, k, :], d_wout[:, k, :])

            nc.gpsimd.memset(xt_lhs[32:33, :], 1.0)
            nc.gpsimd.memset(state[:, :], 0.0)
            nc.gpsimd.memset(st2T[:, :, :], 0.0)

            # ---------- precompute w1^T acts and z0 ----------
            for m in range(HK):
                ps_w = pp1.tile([128, BT], f32, tag="A")
                for k in range(HK):
                    MM(ps_w[:, :], w1tw[:, k, 128 * m:128 * m + 128], encT[:, k, :],
                       start=(k == 0), stop=(k == HK - 1))
                ACT(w1ta[:, m, :, :], ps_w[:, :], AF.Copy)
            for m in range(HK):
                ps_z = pp1.tile([128, BT], f32, tag="C")
                for k in range(HK):
                    MM(ps_z[:, :], w12t[:, k, 128 * m:128 * m + 128], encT[:, k, :],
                       start=(k == 0), stop=(k == HK - 1))
                ACT(zt[:, m, :], ps_z[:, :], AF.Tanh)

            sa_tiles = {}

            def attend_A(j, with_q2):
                """q2^T -> z^T -> u -> softmax -> local partial -> AG trigger."""
                if with_q2:
                    ps_q2 = pp1.tile([128, HK, B], f32, tag="small")
                    for m in range(HK):
                        for k in range(HK):
                            MM(ps_q2[:, m, :], w2t[:, k, 128 * m:128 * m + 128],
                               st2T[:, k, :], start=(k == 0), stop=(k == HK - 1))
                    zs = sp.tile([128, HK, TL, B], f32, tag="zsum")
                    ps_u = pp1.tile([128, 2, H], f32, tag="A")
                    for h2 in range(2):
                        a1, a2 = broadcast_tensor_aps(
                            w1ta[:, 2 * h2:2 * h2 + 2, :, :],
                            ps_q2[:, 2 * h2:2 * h2 + 2, None, :])
                        nc.vector.tensor_tensor(
                            zs[:, 2 * h2:2 * h2 + 2, :, :], a1, a2, OP.add)
                        ACT(zt[:, 2 * h2:2 * h2 + 2, :],
                            zs[:, 2 * h2:2 * h2 + 2, :, :], AF.Tanh)
                        for m2 in range(2):
                            for k in (2 * h2, 2 * h2 + 1):
                                MM(ps_u[:, m2, :],
                                   zt[:, k, 128 * m2:128 * m2 + 128],
                                   vtt[:, k, :], start=(k == 0),
                                   stop=(k == HK - 1))
                if not with_q2:
                    ps_u = pp1.tile([128, 2, H], f32, tag="A")
                    for m2 in range(2):
                        for k in range(HK):
                            MM(ps_u[:, m2, :],
                               zt[:, k, 128 * m2:128 * m2 + 128],
                               vtt[:, k, :], start=(k == 0),
                               stop=(k == HK - 1))
                prod = sp.tile([128, 2, H], bf16, tag="prod")
                for m2 in range(2):
                    aw = sp.tile([128, H], bf16, tag="aw")
                    den = sp.tile([128, 1], f32, tag="den")
                    ACT(aw[:, :], ps_u[:, m2, :], AF.Exp,
                        accum_out=den[:, :])
                    rden = sp.tile([128, 1], f32, tag="rden")
                    nc.vector.reciprocal(rden[:, :], den[:, :])
                    nc.vector.scalar_tensor_tensor(
                        prod[:, m2, :], aw[:, :], rden[:, :], encbt[:, m2, :],
                        OP.mult, OP.mult)
                ps_loc = pp1.tile([B, H], f32, tag="C")
                for kt in range(2):
                    MM(ps_loc[:, :], maskl[:, kt, :], prod[:, kt, :],
                       start=(kt == 0), stop=(kt == 1))
                part = sp.tile([B, H], bf16, tag="part")
                ACT(part[:, :], ps_loc[:, :], AF.Copy)
                s = j % 2
                nc.sync.dma_start(bnc_in[s].ap(), part[:, :])
                nc.gpsimd.collective_compute(
                    "AllGather", OP.bypass, replica_groups=rg,
                    ins=[bnc_in[s].ap()], outs=[bnc_out[s].ap()])


            def attend_B(j):
                """AG result -> gathered shards into SBUF. saT for the wout of
                step j-1 is built directly: saT = st2T + attns^T, where
                attns^T comes from mask-matmuls on the gathered shards
                (st2T still holds state2 of step j-1 at this point)."""
                s = j % 2
                for q in range(4):
                    [nc.sync, nc.sync, nc.gpsimd, nc.gpsimd][q].dma_start(
                        gath[:, :, 128 * q:128 * q + 128],
                        bnc_out[s][:, :, 128 * q:128 * q + 128])
                if j >= 1:
                    ps_aT = pp1.tile([128, HK, B], f32, tag="small")
                    for m in range(HK):
                        for i2 in range(2):
                            MM(ps_aT[:, m, :],
                               gath[:, i2, 128 * m:128 * m + 128],
                               maskg[:, i2, :], start=(i2 == 0),
                               stop=(i2 == 1))
                    saT = sp.tile([128, HK, B], bf16, tag="saT")
                    sa_tiles[j - 1] = saT
                    nc.vector.tensor_add(saT[:, :, :], st2T[:, :, :],
                                         ps_aT[:, :, :])

            def mm_rz(ps_rz, lhs, w, coff, start, stop):
                for g in range(2):
                    for k in range(HK):
                        MM(ps_rz[:, g, :], lhs[:, k, :],
                           w[:, k, coff + 512 * g:coff + 512 * g + 512],
                           start=(start and k == 0), stop=(stop and k == HK - 1))

            def mm_n(ps_n, sl, lhs, w, coff, start, stop):
                for k in range(HK):
                    MM(ps_n[:, sl, :], lhs[:, k, :], w[:, k, coff:coff + 512],
                       start=(start and k == 0), stop=(stop and k == HK - 1))

            def gru_gh(L, hT, whh):
                """State-side GRU matmuls - independent of the pending AG."""
                rz_tag = "A" if L == 0 else "C"
                ps_rz = pp1.tile([B, 2, H], f32, tag=rz_tag)
                ps_n = pp1.tile([B, 2, H], f32, tag="B")  # 0: i_n, 1: h_n
                mm_rz(ps_rz, hT, whh, 0, True, False)
                mm_n(ps_n, 1, hT, whh, 1024, True, True)
                return ps_rz, ps_n

            def gru_gates(ps_rz, ps_n, h_prev, s_out, on_half=None):
                # two h-halves pipelined across ACT and DVE
                HH = H // 2
                rz_s = sp.tile([B, 2, H], f32, tag="gsig")
                zc = sp.tile([B, H], f32, tag="gzc")
                t1 = sp.tile([B, H], f32, tag="gtmp")
                t2 = sp.tile([B, H], f32, tag="gtmp2")
                p1 = sp.tile([B, H], f32, tag="gp1")
                n_s = sp.tile([B, H], f32, tag="gn")
                p2 = sp.tile([B, H], f32, tag="gp2")
                for h in range(2):
                    hs = slice(HH * h, HH * h + HH)
                    ACT(rz_s[:, 0, hs], ps_rz[:, 0, hs], AF.Sigmoid)
                    nc.vector.tensor_mul(t1[:, hs], rz_s[:, 0, hs],
                                         ps_n[:, 1, hs])
                    ACT(rz_s[:, 1, hs], ps_rz[:, 1, hs], AF.Sigmoid)
                    nc.vector.tensor_add(t2[:, hs], ps_n[:, 0, hs], t1[:, hs])
                    ACT(zc[:, hs], ps_rz[:, 1, hs], AF.Sigmoid, scale=-1.0)
                    nc.vector.tensor_mul(p1[:, hs], rz_s[:, 1, hs],
                                         h_prev[:, hs])
                    ACT(n_s[:, hs], t2[:, hs], AF.Tanh)
                    nc.vector.tensor_mul(p2[:, hs], zc[:, hs], n_s[:, hs])
                    nc.vector.tensor_add(s_out[:, hs], p1[:, hs], p2[:, hs])
                    if on_half is not None:
                        on_half(h)

            def transpose_half(dst, src_t, h):
                ps_T = pp1.tile([128, 2, B], f32, tag="small")
                for i, k in enumerate((2 * h, 2 * h + 1)):
                    TP(ps_T[:, i, :], src_t[:, 128 * k:128 * k + 128],
                       id32[0:B, 0:B])
                ACT(dst[:, 2 * h:2 * h + 2, :], ps_T[:, :, :], AF.Copy)

            def wout_chunks(t, saT, chunks):
                for c in chunks:
                    ps_o = pp2.tile([B, 500], f32, tag="out")
                    for k in range(HK):
                        MM(ps_o[:, :], saT[:, k, :],
                           wout[:, k, 500 * c:500 * c + 500],
                           start=(k == 0), stop=(k == HK - 1))
                    o_sb = sp.tile([B, 500], bf16, tag="osb")
                    if c % 2 == 0:
                        nc.vector.tensor_copy(o_sb[:, :], ps_o[:, :])
                    else:
                        ACT(o_sb[:, :], ps_o[:, :], AF.Copy)
                    nc.scalar.dma_start(
                        d_out[32 * t:32 * t + 32, 500 * c:500 * c + 500],
                        o_sb[:, :])

            # ---------- initial attention (attns0) ----------
            attend_A(0, with_q2=False)
            attend_B(0)

            # ---------- decode steps ----------
            # gh1 of step 0 (zero state): filler emitted before attend_B(0)
            g1 = gru_gh(0, st2T, whh1)
            attend_B(0)
            saT_prev = None
            for t in range(TDEC):
                # x^T directly from the gathered shards:
                #   xT[h,b] = sum_r gath[r,h] * WrR^T[r%32,b] + Xl[b,t]
                ps_xt = pp1.tile([128, HK, B], f32, tag="small")
                for m in range(HK):
                    for i2 in range(2):
                        MM(ps_xt[:, m, :], gath[:, i2, 128 * m:128 * m + 128],
                           rhsg[:, i2, :], start=(i2 == 0), stop=False)
                    MM(ps_xt[:, m, :], xt_lhs[32:33, 128 * m:128 * m + 128],
                       rhsx[32:33, t, :], start=False, stop=True)
                for m in range(HK):
                    nc.vector.tensor_copy(xt_sb[:, m, :], ps_xt[:, m, :])

                s1 = sp.tile([B, H], f32, tag="st1")
                mm_rz(g1[0], xt_sb, wih, 0, False, True)
                mm_n(g1[1], 0, xt_sb, wih, 1024, True, True)
                # layer-2 x-side matmuls (fill the gates1 window)
                ps2_rz = pp1.tile([B, 2, H], f32, tag="C")
                ps2_n = pp1.tile([B, 2, H], f32, tag="B")
                mm_rz(ps2_rz, xt_sb, wih, 1536, True, False)
                mm_n(ps2_n, 0, xt_sb, wih, 1536 + 1024, True, True)

                def after1(h):
                    # as each h-half of state1 lands: transpose it and start
                    # the h-side layer-2 matmuls for those k-tiles
                    transpose_half(st1T, s1, h)
                    for g in range(2):
                        for k in (2 * h, 2 * h + 1):
                            MM(ps2_rz[:, g, :], st1T[:, k, :],
                               whh2[:, k, 512 * g:512 * g + 512],
                               start=False, stop=(k == HK - 1))
                    for k in (2 * h, 2 * h + 1):
                        MM(ps2_n[:, 1, :], st1T[:, k, :], whh2[:, k, 1024:1536],
                           start=(k == 0), stop=(k == HK - 1))

                gru_gates(g1[0], g1[1], state, s1, on_half=after1)

                if t >= 1:
                    saT_prev = sa_tiles.pop(t - 1)
                    wout_chunks(t - 1, saT_prev, [0, 1, 2, 3])

                def after2(h):
                    transpose_half(st2T, state, h)

                gru_gates(ps2_rz, ps2_n, s1, state, on_half=after2)

                attend_A(t + 1, with_q2=True)
                # AG in flight: next step's gh1 + second half of wout t-1
                if t < TDEC - 1:
                    g1 = gru_gh(0, st2T, whh1)
                attend_B(t + 1)

            saT = sa_tiles.pop(TDEC - 1)
            wout_chunks(TDEC - 1, saT, [0, 1, 2, 3])
            nc.sync.dma_start(d_state.ap(), state[:, :])

    nc.compile()
    return nc


def _get_built():
    global _BUILT
    if _BUILT is None:
        _BUILT = _build()
    return _BUILT


def _prep(inputs):
    import ml_dtypes
    bf = ml_dtypes.bfloat16

    def f(x):
        return np.asarray(x, np.float32)

    enc = f(inputs["encoder_output"])            # [B, T, H]
    dec = f(inputs["decoder_input"])             # [B, TDEC]
    W1, W2, vT = f(inputs["W1"]), f(inputs["W2"]), f(inputs["vT"])
    Wr, br = f(inputs["Wr"]), f(inputs["br"])
    Wih1, Whh1 = f(inputs["Wih1"]), f(inputs["Whh1"])
    Wih2, Whh2 = f(inputs["Wih2"]), f(inputs["Whh2"])
    Wout = f(inputs["Wout"])

    def sb_layout(M):  # [512, X] -> [128, 4, X]
        X = M.shape[1]
        return np.ascontiguousarray(
            M.reshape(HK, 128, X).transpose(1, 0, 2)).astype(bf)

    w12t = sb_layout((W1 + W2).T)
    w1t = sb_layout(W1.T)
    vtt = sb_layout(vT.T)
    w2t = sb_layout(W2.T)
    wihT = sb_layout(np.concatenate([Wih1.T, Wih2.T], axis=1))
    whh1T = sb_layout(Whh1.T)
    whh2T = sb_layout(Whh2.T)

    # rhs for x^T matmul: rows 0-31 Wr_r^T, row 32 = Xl + br
    Xl = Wr[:, :B] @ dec + br[:, None]           # [B, TDEC]
    rhsx = np.zeros((33, TDEC, B), np.float32)
    rhsx[:B] = np.broadcast_to(Wr[:, B:].T[:, None, :], (B, TDEC, B))
    rhsx[B] = Xl.T
    rhsx = rhsx.astype(bf)

    # masks
    maskl = np.zeros((128, 2, B), np.float32)
    for kt in range(2):
        for k in range(128):
            maskl[k, kt, k % B] = 1.0
    maskg = np.zeros((128, 2, B), np.float32)
    rhsg = np.zeros((128, 2, B), np.float32)
    WrRT = Wr[:, B:].T                           # [k, b]
    for i2 in range(2):
        for k in range(128):
            maskg[k, i2, (2 * k + i2) % B] = 1.0
            rhsg[k, i2, :] = WrRT[(2 * k + i2) % B, :]
    maskl = maskl.astype(bf)
    maskg = maskg.astype(bf)
    rhsg = rhsg.astype(bf)
    id32 = np.eye(128, dtype=np.float32)
    id16 = np.eye(128, dtype=np.float32).astype(bf)

    shared = dict(w12t=w12t, w1t=w1t, vtt=vtt, w2t=w2t, wih=wihT,
                  whh1=whh1T, whh2=whh2T, rhsx=rhsx, maskl=maskl,
                  maskg=maskg, rhsg=rhsg, id32=id32, id16=id16)

    in_maps = []
    for c in range(NCORES):
        enc_sh = enc[:, TL * c:TL * c + TL, :]               # [B, TL, H]
        # t-major local rows: bt = tl*B + b
        encT = sb_layout(enc_sh.transpose(2, 1, 0).reshape(H, BT))
        encbt = np.ascontiguousarray(
            enc_sh.transpose(1, 0, 2).reshape(BT, H)
            .reshape(2, 128, H).transpose(1, 0, 2)
        ).astype(bf)
        woutT = sb_layout(Wout.T[:, VS * c:VS * c + VS])
        m = dict(shared)
        m.update(encT=encT, encbt=encbt, wout=woutT)
        in_maps.append(m)
    return in_maps


def _run(inputs, trace=False, **kw):
    from concourse import bass_utils
    nc = _get_built()
    in_maps = _prep(inputs)
    res = bass_utils.run_bass_kernel_spmd(
        nc, in_maps, core_ids=list(range(NCORES)), trace=trace, **kw)
    outs = [np.asarray(r["out"], np.float32).reshape(TDEC, B, VS)
            for r in res.results]
    out = np.concatenate(outs, axis=2)
    out = out + np.asarray(inputs["bout"], np.float32)[None, None, :]
    st = np.asarray(res.results[0]["state_out"], np.float32)
    return (out, st), res


def kernel(**inputs):
    (out, st), _ = _run(inputs)
    return out, st
